# revision 9
# baseline (speedup 1.0000x reference)
import sys
sys.path.insert(0, "/opt/trn_rl_repo")
import numpy as np
import concourse.bass as bass
import concourse.bacc as bacc
import concourse.tile as tile
from concourse import mybir
from concourse.bass_utils import run_bass_kernel_spmd
from concourse import bass_isa

F32 = mybir.dt.float32
F32R = mybir.dt.float32r
BF16 = mybir.dt.bfloat16
AF = mybir.ActivationFunctionType
OP = mybir.AluOpType

N, CIO, L = 16, 512, 384
DIMHEAD, NUMHEAD, MAXEMBED, DIMGROUP = 64, 8, 384, 8
CHID = DIMHEAD * NUMHEAD
NCORES = 8
BPC = N // NCORES      # batches per core
KC = CIO // 128        # contraction chunks
OC = CHID // 128       # output-channel chunks
JC = L // 128          # key-position chunks

# constant-blob column layout (128-partition blob, fp32r-typed)
CB_I128 = 0            # [128,128] identity
CB_SEL8 = 128          # [128,64] head-selector columns
CB_VRES = 192          # [128,4]
CB_QKO = 196           # [128,4]
CB_QKP = 200           # [128,4]
CB_DB = 204            # [128,4] dense bias
CB_ONES = 208          # [128,64] all-ones
CB_COLS = 272
# 8-partition blob
B8_BSEL = 0            # [8,512]
B8_I8 = 512            # [8,8]
B8_GB = 520            # [8,1] gate bias
B8_COLS = 521


def build_nc(iters=1):
    nc = bacc.Bacc("TRN2", target_bir_lowering=False, debug=False)

    def inp(name, shape, dt):
        return nc.dram_tensor(name, shape, dt, kind="ExternalInput").ap()

    x = inp("x", [BPC, CIO, L], F32R)
    xorg = inp("xorg", [BPC, CIO, L], F32)
    abspos = inp("abspos", [BPC, CIO, L], F32)
    mblob = inp("mblob", [BPC, 128, 4], F32)     # [maskT(3) | norm(1)]
    rptin = inp("rptin", [128, JC * L], BF16)
    rptsw = inp("rptsw", [128, JC * L], BF16)    # partition-halves swapped
    i128b = inp("i128b", [128, 128], BF16)
    iswapb = inp("iswapb", [128, 128], BF16)     # anti-diagonal I64 blocks
    gate_wT = inp("gate_wT", [CIO, NUMHEAD], F32R)
    q_wT = inp("q_wT", [CIO, CHID], F32R)
    k_wT = inp("k_wT", [CIO, CHID], F32R)
    v_wT = inp("v_wT", [CIO, CHID], F32R)
    dense_wT = inp("dense_wT", [CHID, CIO], F32R)
    cblob = inp("cblob", [128, CB_COLS], F32R)
    blob8 = inp("blob8", [NUMHEAD, B8_COLS], F32R)
    out = nc.dram_tensor("out", [BPC, CIO, L], F32, kind="ExternalOutput").ap()

    def chunked_src(t, b):
        # [CIO, L] DRAM slice viewed as [p(128), kc(4), l(384)]
        return bass.AP(tensor=t.tensor, offset=b * CIO * L,
                       ap=[[L, 128], [128 * L, KC], [1, L]])

    def wsrc(t):
        # [CIO, CHID] viewed as [p(128), kc(4), c(CHID)]
        return bass.AP(tensor=t.tensor, offset=0,
                       ap=[[CHID, 128], [128 * CHID, KC], [1, CHID]])

    with tile.TileContext(nc) as tc:
        with tc.tile_pool(name="wts", bufs=1) as wp, \
             tc.tile_pool(name="a2", bufs=2) as a2, \
             tc.tile_pool(name="a1", bufs=1) as a1, \
             tc.tile_pool(name="pp", bufs=12) as ppool, \
             tc.tile_pool(name="ps_big", bufs=3, space="PSUM") as psb, \
             tc.tile_pool(name="ps_s", bufs=3, space="PSUM") as pss, \
             tc.tile_pool(name="ps_o", bufs=2, space="PSUM") as pso:

            first = [True]
            weights = {}

            for it in range(iters):
                binp = []

                def emit_inp(b, ab_engine=None):
                    txa = a1.tile([128, KC * L], F32R, tag=f"xa{b}", name=f"xa{b}")
                    nc.sync.dma_start(out=txa, in_=chunked_src(x, b))
                    txo = a1.tile([128, KC * L], F32, tag=f"xo{b}", name=f"xo{b}")
                    nc.sync.dma_start(out=txo, in_=chunked_src(xorg, b))
                    tab = a1.tile([128, KC * L], F32, tag=f"ab{b}", name=f"ab{b}")
                    (ab_engine or nc.gpsimd).dma_start(out=tab,
                                                       in_=chunked_src(abspos, b))
                    binp.append((txa, txo, tab))

                def wdma(nm, shape, dt, srcap):
                    t = wp.tile([128, shape], dt, tag=nm, name=nm)
                    nc.scalar.dma_start(out=t, in_=srcap)
                    weights[nm] = t
                    return t

                if first[0]:
                    w = weights
                    emit_inp(0)
                    t = wp.tile([NUMHEAD, B8_COLS], F32R, tag="b8", name="b8")
                    nc.scalar.dma_start(out=t, in_=blob8)
                    w["b8"] = t
                    wdma("wg", KC * NUMHEAD, F32R,
                         bass.AP(tensor=gate_wT.tensor, offset=0,
                                 ap=[[NUMHEAD, 128], [128 * NUMHEAD, KC],
                                     [1, NUMHEAD]]))
                    wdma("cb", CB_COLS, F32R, cblob)
                    w["mb"] = []
                    for b in range(BPC):
                        t = wp.tile([128, 4], F32, tag=f"mb{b}", name=f"mb{b}")
                        nc.scalar.dma_start(out=t, in_=mblob[b])
                        w["mb"].append(t)
                    wdma("wq", KC * CHID, F32R, wsrc(q_wT))
                    wdma("wk", KC * CHID, F32R, wsrc(k_wT))
                    wdma("i128b", 128, BF16, i128b)
                    wdma("iswapb", 128, BF16, iswapb)
                    rt = wdma("rptw", JC * L, BF16, rptin)
                    w["rpt"] = [rt[:, jc * L:(jc + 1) * L] for jc in range(JC)]
                    rs = wdma("rptsww", JC * L, BF16, rptsw)
                    w["rptsw"] = [rs[:, jc * L:(jc + 1) * L] for jc in range(JC)]
                    wdma("wv", KC * CHID, F32R, wsrc(v_wT))
                    wdma("wd", KC * CHID, F32R, wsrc(dense_wT))
                    emit_inp(1, ab_engine=nc.sync)
                    first[0] = False
                else:
                    emit_inp(0)
                    emit_inp(1)
                w = weights
                cb, b8 = w["cb"], w["b8"]

                for b in range(BPC):
                    txa, txo, tab = binp[b]
                    xs = lambda t, kc: t[:, kc * L:(kc + 1) * L]

                    rn = a2.tile([128, 1], F32, tag="rn", name="rn")
                    nc.vector.reciprocal(rn, w["mb"][b][:, 3:4])

                    # ---- x0 = x + vres*xorg ; x1 = x + qko*xorg + qkp*abspos ----
                    tx0 = a1.tile([128, KC * L], F32R, tag="x0", name="x0")
                    tx1 = a1.tile([128, KC * L], F32R, tag="x1", name="x1")
                    ttm = a1.tile([128, KC * L], F32, tag="tt", name="tt")
                    tmv = a1.tile([128, KC * L], F32, tag="tmv", name="tmv")

                    def bcast1(col):
                        sv = cb[:, col:col + 1].bitcast(F32)
                        return bass.AP(tensor=sv.tensor, offset=sv.offset,
                                       ap=[sv.ap[0], [0, L]])

                    # x1 first (gates q/k projections): DVE fused ops
                    for kc in range(KC):
                        nc.vector.scalar_tensor_tensor(
                            xs(ttm, kc), xs(txo, kc),
                            cb[:, CB_QKO + kc:CB_QKO + kc + 1].bitcast(F32),
                            xs(txa, kc).bitcast(F32), OP.mult, OP.add)
                        nc.vector.scalar_tensor_tensor(
                            xs(tx1, kc), xs(tab, kc),
                            cb[:, CB_QKP + kc:CB_QKP + kc + 1].bitcast(F32),
                            xs(ttm, kc), OP.mult, OP.add)
                    # x0 (feeds vT later): Pool, off the critical path
                    for kc in range(KC):
                        nc.gpsimd.tensor_tensor(out=xs(tmv, kc), in0=xs(txo, kc),
                                                in1=bcast1(CB_VRES + kc), op=OP.mult)
                        nc.gpsimd.tensor_tensor(out=xs(tx0, kc), in0=xs(tmv, kc),
                                                in1=xs(txa, kc).bitcast(F32),
                                                op=OP.add)

                    # ---- gate projection (PE) ----
                    g_ps = psb.tile([NUMHEAD, L], F32, tag="big", name="g_ps")
                    for kc in range(KC):
                        nc.tensor.matmul(
                            g_ps, w["wg"][:, kc * NUMHEAD:(kc + 1) * NUMHEAD],
                            xs(txa, kc), start=(kc == 0), stop=(kc == KC - 1))
                    tgate = a2.tile([NUMHEAD, L], F32R, tag="gate", name="gate")
                    nc.scalar.activation(tgate, g_ps, AF.Identity,
                                         bias=b8[:, B8_GB:B8_GB + 1].bitcast(F32))

                    # ---- q/k projections (PE busy while gate transposes) ----
                    def proj(wall, rhs_all, nm):
                        dst = a2.tile([128, OC * L], F32R, tag=nm, name=nm)
                        for o in range(OC):
                            ps = psb.tile([128, L], F32, tag="big",
                                          name=f"{nm}p{o}")
                            for kc in range(KC):
                                nc.tensor.matmul(
                                    ps,
                                    wall[:, kc * CHID + o * 128:
                                         kc * CHID + o * 128 + 128],
                                    xs(rhs_all, kc),
                                    start=(kc == 0), stop=(kc == KC - 1))
                            if o == 0:
                                nc.scalar.copy(dst[:, o * L:(o + 1) * L], ps)
                            else:
                                nc.vector.tensor_copy(dst[:, o * L:(o + 1) * L], ps)
                        return dst

                    tgm = []

                    def emit_gt():
                        for jc in range(JC):
                            gt_ps = psb.tile([128, NUMHEAD], F32, tag="big",
                                             name=f"gt{jc}")
                            nc.tensor.matmul(gt_ps,
                                             tgate[:, jc * 128:(jc + 1) * 128],
                                             b8[:, B8_I8:B8_I8 + NUMHEAD],
                                             start=True, stop=True)
                            gm = a2.tile([128, NUMHEAD], F32, tag=f"gm{jc}",
                                         name=f"gm{jc}")
                            nc.vector.tensor_scalar(gm, gt_ps,
                                                    w["mb"][b][:, jc:jc + 1],
                                                    rn[:, 0:1], OP.add, OP.mult)
                            tgm.append(gm)

                    tq = a2.tile([128, OC * L], F32R, tag="q", name="q")
                    for o in range(OC):
                        ps = psb.tile([128, L], F32, tag="big", name=f"qp{o}")
                        for kc in range(KC):
                            nc.tensor.matmul(
                                ps,
                                w["wq"][:, kc * CHID + o * 128:
                                        kc * CHID + o * 128 + 128],
                                xs(tx1, kc),
                                start=(kc == 0), stop=(kc == KC - 1))
                        if o == 0:
                            nc.scalar.copy(tq[:, o * L:(o + 1) * L], ps)
                        else:
                            nc.vector.tensor_copy(tq[:, o * L:(o + 1) * L], ps)
                    emit_gt()

                    tk = proj(w["wk"], tx1, "k")

                    # ---- vT = x0^T @ v_wT ; per-head 65-col blocks, col 64 = ones ----
                    VW = DIMHEAD + 1
                    tvt = a2.tile([128, JC * NUMHEAD * VW], F32R, tag="vt", name="vt")

                    for lc in range(JC):
                        ps = psb.tile([128, CHID], F32, tag="big", name=f"vtp{lc}")
                        for kc in range(KC):
                            nc.tensor.matmul(
                                ps, tx0[:, kc * L + lc * 128:kc * L + lc * 128 + 128],
                                w["wv"][:, kc * CHID:(kc + 1) * CHID],
                                start=(kc == 0), stop=(kc == KC - 1))
                        blk = tvt[:, lc * NUMHEAD * VW:(lc + 1) * NUMHEAD * VW]
                        blk = blk.rearrange("p (h c) -> p h c", c=VW)
                        nc.vector.tensor_copy(blk[:, :, 0:DIMHEAD], ps)
                        nc.vector.tensor_copy(blk[:, :, DIMHEAD:VW],
                                              cb[:, CB_ONES:CB_ONES + NUMHEAD])

                    # ---- attention (emission software-pipelined over head pairs) ----
                    # QK pairs run concurrently on disjoint PE row groups;
                    # relpos pre-fill uses 4 quadrant-disjoint 64-row matmuls
                    # (rpt + partition-swapped rptsw) so all 4 overlap on HW.
                    tP_heads = {}
                    tatt = a1.tile([128, OC * L], F32R, tag="att", name="att")

                    def emit_qk_pair(hp):
                        he, ho = 2 * hp, 2 * hp + 1
                        tPe, tPo = [], []
                        for jc in range(JC):
                            s_e = pss.tile([128, L], F32, tag="s",
                                           name=f"s{he}j{jc}")
                            s_o = pss.tile([128, L], F32, tag="s",
                                           name=f"s{ho}j{jc}")
                            rpt, rsw = w["rpt"][jc], w["rptsw"][jc]
                            ib, isw = w["i128b"], w["iswapb"]
                            nc.tensor.matmul(s_e[0:64, :], ib[0:64, 0:64],
                                             rpt[0:64, :], start=True, stop=False,
                                             skip_group_check=True)
                            nc.tensor.matmul(s_e[64:128, :], ib[64:128, 64:128],
                                             rpt[64:128, :], start=True, stop=False,
                                             skip_group_check=True)
                            nc.tensor.matmul(s_o[0:64, :], isw[64:128, 0:64],
                                             rsw[64:128, :], start=True, stop=False,
                                             skip_group_check=True)
                            nc.tensor.matmul(s_o[64:128, :], isw[0:64, 64:128],
                                             rsw[0:64, :], start=True, stop=False,
                                             skip_group_check=True)
                            co = hp * L + jc * 128
                            nc.tensor.matmul(s_e, tk[0:64, co:co + 128],
                                             tq[0:64, hp * L:(hp + 1) * L],
                                             start=False, stop=True,
                                             skip_group_check=True)
                            nc.tensor.matmul(s_o, tk[64:128, co:co + 128],
                                             tq[64:128, hp * L:(hp + 1) * L],
                                             start=False, stop=True,
                                             skip_group_check=True)
                            Pe = ppool.tile([128, L], F32R, tag="P",
                                            name=f"P{he}j{jc}")
                            nc.scalar.activation(Pe, s_e, AF.Exp,
                                                 bias=tgm[jc][:, he:he + 1],
                                                 scale=rn[:, 0:1])
                            Po = ppool.tile([128, L], F32R, tag="P",
                                            name=f"P{ho}j{jc}")
                            nc.scalar.activation(Po, s_o, AF.Exp,
                                                 bias=tgm[jc][:, ho:ho + 1],
                                                 scale=rn[:, 0:1])
                            tPe.append(Pe)
                            tPo.append(Po)
                        tP_heads[he] = tPe
                        tP_heads[ho] = tPo

                    def emit_av_pair(hp):
                        he, ho = 2 * hp, 2 * hp + 1
                        oUs = {}
                        for h in (he, ho):
                            tP = tP_heads.pop(h)
                            o_ps = pso.tile([VW, L], F32, tag="o", name=f"o{h}")
                            for jc in range(JC):
                                nc.tensor.matmul(
                                    o_ps,
                                    tvt[:, jc * NUMHEAD * VW + VW * h:
                                        jc * NUMHEAD * VW + VW * h + VW],
                                    tP[jc], start=(jc == 0), stop=(jc == JC - 1),
                                    skip_group_check=True)
                            oU = a1.tile([VW, L], F32, tag=f"oU{h}", name=f"oU{h}")
                            if h >= NUMHEAD - 2:
                                nc.scalar.copy(oU, o_ps)
                            else:
                                nc.vector.tensor_copy(oU, o_ps)
                            rcpr65 = a2.tile([VW, L], F32R, tag="rcpr65",
                                             name=f"rr{h}")
                            with nc.allow_low_precision(reason="fp32r recip"):
                                nc.vector.reciprocal(rcpr65[64:65, :], oU[64:65, :])
                            oUs[h] = (oU, rcpr65)
                        for h, off in ((he, 0), (ho, 64)):
                            rb_ps = psb.tile([64, L], F32, tag="big",
                                             name=f"rb{h}")
                            nc.tensor.matmul(rb_ps,
                                             cb[64:65, CB_ONES:CB_ONES + 64],
                                             oUs[h][1][64:65, :], start=True,
                                             stop=True, skip_group_check=True)
                            nc.vector.tensor_tensor(
                                out=tatt[off:off + 64, hp * L:(hp + 1) * L],
                                in0=oUs[h][0][0:64, :], in1=rb_ps,
                                op=OP.mult)

                    emit_qk_pair(0)
                    tout = a2.tile([128, OC * L], F32, tag="outt", name="outt")
                    d_ps = {}

                    def emit_dense_partial(o, kcs, stop):
                        if o not in d_ps:
                            d_ps[o] = psb.tile([128, L], F32, tag="big",
                                               name=f"dp{o}")
                        for kc in kcs:
                            nc.tensor.matmul(
                                d_ps[o],
                                w["wd"][:, kc * CIO + o * 128:kc * CIO + o * 128 + 128],
                                tatt[:, kc * L:(kc + 1) * L],
                                start=(kc == 0), stop=(stop and kc == kcs[-1]),
                                skip_group_check=True)
                        if stop:
                            nc.scalar.activation(
                                tout[:, o * L:(o + 1) * L], d_ps.pop(o), AF.Identity,
                                bias=cb[:, CB_DB + o:CB_DB + o + 1].bitcast(F32))

                    for hp in range(NUMHEAD // 2):
                        if hp + 1 < NUMHEAD // 2:
                            emit_qk_pair(hp + 1)
                        emit_av_pair(hp)
                        if hp == 2:
                            for o in (0, 1):
                                emit_dense_partial(o, [0, 1, 2], stop=False)
                    for o in (0, 1):
                        emit_dense_partial(o, [3], stop=True)
                    for o in (2, 3):
                        emit_dense_partial(o, [0, 1, 2, 3], stop=True)
                    for half in range(2):
                        dst = bass.AP(tensor=out.tensor,
                                      offset=b * CIO * L + half * 2 * 128 * L,
                                      ap=[[L, 128], [128 * L, 2], [1, L]])
                        nc.sync.dma_start(
                            out=dst, in_=tout[:, half * 2 * L:(half + 1) * 2 * L])

    nc.compile()
    return nc


_CACHE = {}


def _get_nc(iters=1):
    if iters not in _CACHE:
        _CACHE[iters] = build_nc(iters)
    return _CACHE[iters]


def _bf16(a):
    import ml_dtypes
    return np.asarray(a, dtype=ml_dtypes.bfloat16)


def _make_rpt(relpos):
    # rpt[p, jc*L + i] = relpos[clip(384 + 128*jc + p - i, 0, 766)]
    j = np.arange(L)[:, None]
    i = np.arange(L)[None, :]
    idx = np.clip(MAXEMBED + j - i, 0, 2 * MAXEMBED - 2)
    rp = relpos[idx]                                  # [j, i]
    return np.ascontiguousarray(
        rp.reshape(JC, 128, L).transpose(1, 0, 2).reshape(128, JC * L))


def _host_prep(inputs):
    f32 = lambda a: np.ascontiguousarray(np.asarray(a), dtype=np.float32)
    x, xorg, abspos = f32(inputs["x"]), f32(inputs["xorg"]), f32(inputs["abspos"])
    mask, norm = f32(inputs["mask"]), f32(inputs["norm"])
    relpos = f32(inputs["relpos"])

    def expand_res(r):
        e = np.repeat(f32(r).reshape(-1), DIMGROUP)          # [512]
        return np.ascontiguousarray(e.reshape(KC, 128).T)    # [128, KC]

    cblob = np.zeros((128, CB_COLS), np.float32)
    cblob[:, CB_I128:CB_I128 + 128] = np.eye(128, dtype=np.float32)
    for h in range(NUMHEAD):
        cblob[:, CB_SEL8 + 8 * h + (h % 2)] = 1.0
    cblob[:, CB_VRES:CB_VRES + KC] = expand_res(inputs["vorg_res"])
    cblob[:, CB_QKO:CB_QKO + KC] = expand_res(inputs["qkorg_res"])
    cblob[:, CB_QKP:CB_QKP + KC] = expand_res(inputs["qkpos_res"])
    cblob[:, CB_DB:CB_DB + KC] = np.ascontiguousarray(
        f32(inputs["dense_b"]).reshape(OC, 128).T)
    cblob[:, CB_ONES:CB_ONES + 64] = 1.0

    blob8 = np.zeros((NUMHEAD, B8_COLS), np.float32)
    for h in range(NUMHEAD):
        blob8[h % 2, B8_BSEL + 64 * h:B8_BSEL + 64 * h + 64] = 1.0
    blob8[:, B8_I8:B8_I8 + NUMHEAD] = np.eye(NUMHEAD, dtype=np.float32)
    blob8[:, B8_GB] = f32(inputs["gate_b"])

    rpt = _make_rpt(relpos)
    iswap = np.zeros((128, 128), np.float32)
    r = np.arange(64)
    iswap[64 + r, r] = 1.0
    iswap[r, 64 + r] = 1.0
    shared = {
        "rptin": _bf16(rpt),
        "rptsw": _bf16(np.concatenate([rpt[64:], rpt[:64]], axis=0)),
        "i128b": _bf16(np.eye(128, dtype=np.float32)),
        "iswapb": _bf16(iswap),
        "gate_wT": np.ascontiguousarray(f32(inputs["gate_w"]).T),
        "q_wT": np.ascontiguousarray(f32(inputs["q_w"]).T),
        "k_wT": np.ascontiguousarray(f32(inputs["k_w"]).T),
        "v_wT": np.ascontiguousarray(f32(inputs["v_w"]).T),
        "dense_wT": np.ascontiguousarray(f32(inputs["dense_w"]).T),
        "cblob": cblob,
        "blob8": blob8,
    }
    # mblob: [N, 128, 4] = [maskT(3) | norm(1)]
    mblob = np.zeros((N, 128, 4), np.float32)
    mblob[:, :, 0:3] = mask.reshape(N, JC, 128).transpose(0, 2, 1)
    mblob[:, :, 3] = norm[:, None]
    in_maps = []
    for c in range(NCORES):
        sl = slice(BPC * c, BPC * (c + 1))
        m = dict(shared)
        m["x"] = x[sl]
        m["xorg"] = xorg[sl]
        m["abspos"] = abspos[sl]
        m["mblob"] = mblob[sl]
        in_maps.append(m)
    return in_maps


def run_on_hw(inputs, iters=1):
    nc = _get_nc(iters)
    in_maps = _host_prep(inputs)
    res = run_bass_kernel_spmd(nc, in_maps, list(range(NCORES)))
    return np.concatenate([res.results[c]["out"] for c in range(NCORES)], axis=0)


def kernel(**inputs) -> np.ndarray:
    return run_on_hw(inputs, iters=1)



# revision 23
# speedup vs baseline: 1.1777x; 1.1777x over previous
import sys
sys.path.insert(0, "/opt/trn_rl_repo")
import numpy as np
import concourse.bass as bass
import concourse.bacc as bacc
import concourse.tile as tile
from concourse import mybir
from concourse.bass_utils import run_bass_kernel_spmd
from concourse import bass_isa

F32 = mybir.dt.float32
F32R = mybir.dt.float32r
BF16 = mybir.dt.bfloat16
AF = mybir.ActivationFunctionType
OP = mybir.AluOpType

N, CIO, L = 16, 512, 384
DIMHEAD, NUMHEAD, MAXEMBED, DIMGROUP = 64, 8, 384, 8
CHID = DIMHEAD * NUMHEAD
NCORES = 8
BPC = N // NCORES      # batches per core
KC = CIO // 128        # contraction chunks
OC = CHID // 128       # output-channel chunks
JC = L // 128          # key-position chunks

# constant-blob column layout (128-partition blob, fp32r-typed)
CB_I128 = 0            # [128,128] identity
CB_SEL8 = 128          # [128,64] head-selector columns
CB_VRES = 192          # [128,4]
CB_QKO = 196           # [128,4]
CB_QKP = 200           # [128,4]
CB_DB = 204            # [128,4] dense bias
CB_ONES = 208          # [128,64] all-ones
CB_COLS = 272
# 8-partition blob
B8_BSEL = 0            # [8,512]
B8_I8 = 512            # [8,8]
B8_GB = 520            # [8,1] gate bias
B8_COLS = 521


def build_nc(iters=1):
    nc = bacc.Bacc("TRN2", target_bir_lowering=False, debug=False)

    def inp(name, shape, dt):
        return nc.dram_tensor(name, shape, dt, kind="ExternalInput").ap()

    x = inp("x", [BPC, CIO, L], BF16)
    xorg = inp("xorg", [BPC, CIO, L], BF16)
    abspos = inp("abspos", [BPC, CIO, L], BF16)
    mblob = inp("mblob", [BPC, 128, 4], F32)     # [maskT(3) | norm(1)]
    rptin = inp("rptin", [128, JC * L], BF16)
    rptsw = inp("rptsw", [128, JC * L], BF16)    # partition-halves swapped
    i128b = inp("i128b", [128, 128], BF16)
    iswapb = inp("iswapb", [128, 128], BF16)     # anti-diagonal I64 blocks
    gate_wT = inp("gate_wT", [CIO, NUMHEAD], BF16)
    q_wT = inp("q_wT", [CIO, CHID], BF16)
    k_wT = inp("k_wT", [CIO, CHID], BF16)
    v_wT = inp("v_wT", [CIO, CHID], BF16)
    dense_wT = inp("dense_wT", [CHID, CIO], BF16)
    cblob = inp("cblob", [128, CB_COLS], F32R)
    blob8 = inp("blob8", [NUMHEAD, B8_COLS], F32R)
    out = nc.dram_tensor("out", [BPC, CIO, L], F32, kind="ExternalOutput").ap()

    def chunked_src(t, b):
        # [CIO, L] DRAM slice viewed as [p(128), kc(4), l(384)]
        return bass.AP(tensor=t.tensor, offset=b * CIO * L,
                       ap=[[L, 128], [128 * L, KC], [1, L]])

    def wsrc(t):
        # [CIO, CHID] viewed as [p(128), kc(4), c(CHID)]
        return bass.AP(tensor=t.tensor, offset=0,
                       ap=[[CHID, 128], [128 * CHID, KC], [1, CHID]])

    with tile.TileContext(nc) as tc:
        with tc.tile_pool(name="wts", bufs=1) as wp, \
             tc.tile_pool(name="a2", bufs=2) as a2, \
             tc.tile_pool(name="a1", bufs=1) as a1, \
             tc.tile_pool(name="pp", bufs=12) as ppool, \
             tc.tile_pool(name="ps_big", bufs=2, space="PSUM") as psb, \
             tc.tile_pool(name="ps_s", bufs=2, space="PSUM") as pss, \
             tc.tile_pool(name="ps_o", bufs=2, space="PSUM") as pso:

            first = [True]
            weights = {}

            for it in range(iters):
                binp = []

                def emit_inp(b, ab_engine=None):
                    txa = a1.tile([128, KC * L], BF16, tag=f"xa{b}", name=f"xa{b}")
                    nc.sync.dma_start(out=txa, in_=chunked_src(x, b))
                    txo = a1.tile([128, KC * L], BF16, tag=f"xo{b}", name=f"xo{b}")
                    nc.sync.dma_start(out=txo, in_=chunked_src(xorg, b))
                    tab = a1.tile([128, KC * L], BF16, tag=f"ab{b}", name=f"ab{b}")
                    (ab_engine or nc.gpsimd).dma_start(out=tab,
                                                       in_=chunked_src(abspos, b))
                    binp.append((txa, txo, tab))

                def wdma(nm, shape, dt, srcap):
                    t = wp.tile([128, shape], dt, tag=nm, name=nm)
                    nc.scalar.dma_start(out=t, in_=srcap)
                    weights[nm] = t
                    return t

                if first[0]:
                    w = weights
                    emit_inp(0)
                    t = wp.tile([NUMHEAD, B8_COLS], F32R, tag="b8", name="b8")
                    nc.scalar.dma_start(out=t, in_=blob8)
                    w["b8"] = t
                    wdma("wg", KC * NUMHEAD, BF16,
                         bass.AP(tensor=gate_wT.tensor, offset=0,
                                 ap=[[NUMHEAD, 128], [128 * NUMHEAD, KC],
                                     [1, NUMHEAD]]))
                    wdma("cb", CB_COLS, F32R, cblob)
                    w["mb"] = []
                    for b in range(BPC):
                        t = wp.tile([128, 4], F32, tag=f"mb{b}", name=f"mb{b}")
                        nc.scalar.dma_start(out=t, in_=mblob[b])
                        w["mb"].append(t)
                    wdma("wq", KC * CHID, BF16, wsrc(q_wT))
                    wdma("wk", KC * CHID, BF16, wsrc(k_wT))
                    wdma("i128b", 128, BF16, i128b)
                    wdma("iswapb", 128, BF16, iswapb)
                    rt = wdma("rptw", JC * L, BF16, rptin)
                    w["rpt"] = [rt[:, jc * L:(jc + 1) * L] for jc in range(JC)]
                    rs = wdma("rptsww", JC * L, BF16, rptsw)
                    w["rptsw"] = [rs[:, jc * L:(jc + 1) * L] for jc in range(JC)]
                    wdma("wv", KC * CHID, BF16, wsrc(v_wT))
                    wdma("wd", KC * CHID, BF16, wsrc(dense_wT))
                    emit_inp(1, ab_engine=nc.sync)
                    first[0] = False
                else:
                    emit_inp(0)
                    emit_inp(1)
                w = weights
                cb, b8 = w["cb"], w["b8"]

                for b in range(BPC):
                    txa, txo, tab = binp[b]
                    xs = lambda t, kc: t[:, kc * L:(kc + 1) * L]

                    rn = a2.tile([128, 1], F32, tag="rn", name="rn")
                    nc.vector.reciprocal(rn, w["mb"][b][:, 3:4])

                    # ---- x0 = x + vres*xorg ; x1 = x + qko*xorg + qkp*abspos ----
                    tx0 = a1.tile([128, KC * L], BF16, tag="x0", name="x0")
                    tx1 = a1.tile([128, KC * L], BF16, tag="x1", name="x1")
                    ttm = a1.tile([128, KC * L], F32, tag="tt", name="tt")
                    tmv = a1.tile([128, KC * L], F32, tag="tmv", name="tmv")

                    def bcast1(col):
                        sv = cb[:, col:col + 1].bitcast(F32)
                        return bass.AP(tensor=sv.tensor, offset=sv.offset,
                                       ap=[sv.ap[0], [0, L]])

                    # x1 first (gates q/k projections): DVE fused ops
                    for kc in range(KC):
                        nc.vector.scalar_tensor_tensor(
                            xs(ttm, kc), xs(txo, kc),
                            cb[:, CB_QKO + kc:CB_QKO + kc + 1].bitcast(F32),
                            xs(txa, kc), OP.mult, OP.add)
                        nc.vector.scalar_tensor_tensor(
                            xs(tx1, kc), xs(tab, kc),
                            cb[:, CB_QKP + kc:CB_QKP + kc + 1].bitcast(F32),
                            xs(ttm, kc), OP.mult, OP.add)
                    # x0 (feeds vT later): Pool, off the critical path
                    for kc in range(KC):
                        nc.gpsimd.tensor_tensor(out=xs(tmv, kc), in0=xs(txo, kc),
                                                in1=bcast1(CB_VRES + kc), op=OP.mult)
                        nc.gpsimd.tensor_tensor(out=xs(tx0, kc), in0=xs(tmv, kc),
                                                in1=xs(txa, kc), op=OP.add)

                    # ---- gate projection (PE) ----
                    g_ps = psb.tile([NUMHEAD, L], F32, tag="big", name="g_ps")
                    for kc in range(KC):
                        nc.tensor.matmul(
                            g_ps, w["wg"][:, kc * NUMHEAD:(kc + 1) * NUMHEAD],
                            xs(txa, kc), start=(kc == 0), stop=(kc == KC - 1))
                    tgate = a2.tile([NUMHEAD, L], F32R, tag="gate", name="gate")
                    nc.scalar.activation(tgate, g_ps, AF.Identity,
                                         bias=b8[:, B8_GB:B8_GB + 1].bitcast(F32))

                    # ---- q/k projections (PE busy while gate transposes) ----
                    def proj(wall, rhs_all, nm):
                        dst = a2.tile([128, OC * L], BF16, tag=nm, name=nm)
                        for o in range(OC):
                            ps = psb.tile([128, L], F32, tag="big",
                                          name=f"{nm}p{o}")
                            for kc in range(KC):
                                nc.tensor.matmul(
                                    ps,
                                    wall[:, kc * CHID + o * 128:
                                         kc * CHID + o * 128 + 128],
                                    xs(rhs_all, kc),
                                    start=(kc == 0), stop=(kc == KC - 1))
                            if o == 0:
                                nc.scalar.copy(dst[:, o * L:(o + 1) * L], ps)
                            else:
                                nc.vector.tensor_copy(dst[:, o * L:(o + 1) * L], ps)
                        return dst

                    tgm = []

                    def emit_gt():
                        for jc in range(JC):
                            gt_ps = psb.tile([128, NUMHEAD], F32, tag="big",
                                             name=f"gt{jc}")
                            nc.tensor.matmul(gt_ps,
                                             tgate[:, jc * 128:(jc + 1) * 128],
                                             b8[:, B8_I8:B8_I8 + NUMHEAD],
                                             start=True, stop=True)
                            gm = a2.tile([128, NUMHEAD], F32, tag=f"gm{jc}",
                                         name=f"gm{jc}")
                            nc.vector.tensor_scalar(gm, gt_ps,
                                                    w["mb"][b][:, jc:jc + 1],
                                                    rn[:, 0:1], OP.add, OP.mult)
                            tgm.append(gm)

                    tq = a2.tile([128, OC * L], BF16, tag="q", name="q")
                    for o in range(OC):
                        ps = psb.tile([128, L], F32, tag="big", name=f"qp{o}")
                        for kc in range(KC):
                            nc.tensor.matmul(
                                ps,
                                w["wq"][:, kc * CHID + o * 128:
                                        kc * CHID + o * 128 + 128],
                                xs(tx1, kc),
                                start=(kc == 0), stop=(kc == KC - 1))
                        if o == 0:
                            nc.scalar.copy(tq[:, o * L:(o + 1) * L], ps)
                        else:
                            nc.vector.tensor_copy(tq[:, o * L:(o + 1) * L], ps)
                    emit_gt()

                    tk = proj(w["wk"], tx1, "k")

                    # ---- vT = x0^T @ v_wT ; per-head 65-col blocks, col 64 = ones ----
                    VW = DIMHEAD + 1
                    tvt = a2.tile([128, JC * NUMHEAD * VW], F32R, tag="vt", name="vt")

                    for lc in range(JC):
                        ps = psb.tile([128, CHID], F32, tag="big", name=f"vtp{lc}")
                        for kc in range(KC):
                            nc.tensor.matmul(
                                ps, tx0[:, kc * L + lc * 128:kc * L + lc * 128 + 128],
                                w["wv"][:, kc * CHID:(kc + 1) * CHID],
                                start=(kc == 0), stop=(kc == KC - 1))
                        blk = tvt[:, lc * NUMHEAD * VW:(lc + 1) * NUMHEAD * VW]
                        blk = blk.rearrange("p (h c) -> p h c", c=VW)
                        nc.vector.tensor_copy(blk[:, :, 0:DIMHEAD], ps)
                        nc.vector.tensor_copy(blk[:, :, DIMHEAD:VW],
                                              cb[:, CB_ONES:CB_ONES + NUMHEAD])

                    # ---- attention (emission software-pipelined over head pairs) ----
                    # QK pairs run concurrently on disjoint PE row groups;
                    # relpos pre-fill uses 4 quadrant-disjoint 64-row matmuls
                    # (rpt + partition-swapped rptsw) so all 4 overlap on HW.
                    tP_heads = {}
                    tatt = a1.tile([128, OC * L], BF16, tag="att", name="att")

                    def emit_qk_pair(hp):
                        he, ho = 2 * hp, 2 * hp + 1
                        tPe, tPo = [], []
                        for jc in range(JC):
                            # one 2-bank psum tile per pair: even head in bank
                            # 0, odd head in bank 1
                            s2 = pss.tile([128, 1024], F32, tag="s",
                                          name=f"s{hp}j{jc}")
                            s_e, s_o = s2[:, 0:L], s2[:, 512:512 + L]
                            rpt, rsw = w["rpt"][jc], w["rptsw"][jc]
                            ib, isw = w["i128b"], w["iswapb"]
                            nc.tensor.matmul(s_e[0:64, :], ib[0:64, 0:64],
                                             rpt[0:64, :], start=True, stop=False,
                                             skip_group_check=True)
                            nc.tensor.matmul(s_e[64:128, :], ib[64:128, 64:128],
                                             rpt[64:128, :], start=True, stop=False,
                                             skip_group_check=True)
                            nc.tensor.matmul(s_o[0:64, :], isw[64:128, 0:64],
                                             rsw[64:128, :], start=True, stop=False,
                                             skip_group_check=True)
                            nc.tensor.matmul(s_o[64:128, :], isw[0:64, 64:128],
                                             rsw[0:64, :], start=True, stop=False,
                                             skip_group_check=True)
                            co = hp * L + jc * 128
                            nc.tensor.matmul(s_e, tk[0:64, co:co + 128],
                                             tq[0:64, hp * L:(hp + 1) * L],
                                             start=False, stop=True,
                                             skip_group_check=True)
                            nc.tensor.matmul(s_o, tk[64:128, co:co + 128],
                                             tq[64:128, hp * L:(hp + 1) * L],
                                             start=False, stop=True,
                                             skip_group_check=True)
                            Pe = ppool.tile([128, L], F32R, tag="P",
                                            name=f"P{he}j{jc}")
                            nc.scalar.activation(Pe, s_e, AF.Exp,
                                                 bias=tgm[jc][:, he:he + 1],
                                                 scale=rn[:, 0:1])
                            Po = ppool.tile([128, L], F32R, tag="P",
                                            name=f"P{ho}j{jc}")
                            nc.scalar.activation(Po, s_o, AF.Exp,
                                                 bias=tgm[jc][:, ho:ho + 1],
                                                 scale=rn[:, 0:1])
                            tPe.append(Pe)
                            tPo.append(Po)
                        tP_heads[he] = tPe
                        tP_heads[ho] = tPo

                    def emit_av_pair(hp):
                        he, ho = 2 * hp, 2 * hp + 1
                        oUs = {}
                        for h in (he, ho):
                            tP = tP_heads.pop(h)
                            o_ps = pso.tile([VW, L], F32, tag="o", name=f"o{h}")
                            for jc in range(JC):
                                nc.tensor.matmul(
                                    o_ps,
                                    tvt[:, jc * NUMHEAD * VW + VW * h:
                                        jc * NUMHEAD * VW + VW * h + VW],
                                    tP[jc], start=(jc == 0), stop=(jc == JC - 1),
                                    skip_group_check=True)
                            oU = a1.tile([VW, L], F32, tag=f"oU{h}", name=f"oU{h}")
                            if h >= NUMHEAD - 2:
                                nc.scalar.copy(oU, o_ps)
                            else:
                                nc.vector.tensor_copy(oU, o_ps)
                            rcpr65 = a2.tile([VW, L], F32R, tag="rcpr65",
                                             name=f"rr{h}")
                            with nc.allow_low_precision(reason="fp32r recip"):
                                nc.vector.reciprocal(rcpr65[64:65, :], oU[64:65, :])
                            oUs[h] = (oU, rcpr65)
                        for h, off in ((he, 0), (ho, 64)):
                            rb_ps = psb.tile([64, L], F32, tag="big",
                                             name=f"rb{h}")
                            nc.tensor.matmul(rb_ps,
                                             cb[64:65, CB_ONES:CB_ONES + 64],
                                             oUs[h][1][64:65, :], start=True,
                                             stop=True, skip_group_check=True)
                            nc.vector.tensor_tensor(
                                out=tatt[off:off + 64, hp * L:(hp + 1) * L],
                                in0=oUs[h][0][0:64, :], in1=rb_ps,
                                op=OP.mult)

                    emit_qk_pair(0)
                    tout = a2.tile([128, OC * L], F32, tag="outt", name="outt")
                    d_ps = {}

                    def emit_dense_partial(o, kcs, stop):
                        if o not in d_ps:
                            d_ps[o] = psb.tile([128, L], F32, tag="big",
                                               name=f"dp{o}")
                        for kc in kcs:
                            nc.tensor.matmul(
                                d_ps[o],
                                w["wd"][:, kc * CIO + o * 128:kc * CIO + o * 128 + 128],
                                tatt[:, kc * L:(kc + 1) * L],
                                start=(kc == 0), stop=(stop and kc == kcs[-1]),
                                skip_group_check=True)
                        if stop:
                            nc.scalar.activation(
                                tout[:, o * L:(o + 1) * L], d_ps.pop(o), AF.Identity,
                                bias=cb[:, CB_DB + o:CB_DB + o + 1].bitcast(F32))

                    for hp in range(NUMHEAD // 2):
                        if hp + 1 < NUMHEAD // 2:
                            emit_qk_pair(hp + 1)
                        emit_av_pair(hp)
                        if hp == 2:
                            emit_dense_partial(0, [0, 1, 2], stop=False)
                    emit_dense_partial(0, [3], stop=True)
                    for o in (1, 2, 3):
                        emit_dense_partial(o, [0, 1, 2, 3], stop=True)
                    for half in range(2):
                        dst = bass.AP(tensor=out.tensor,
                                      offset=b * CIO * L + half * 2 * 128 * L,
                                      ap=[[L, 128], [128 * L, 2], [1, L]])
                        nc.sync.dma_start(
                            out=dst, in_=tout[:, half * 2 * L:(half + 1) * 2 * L])

    nc.compile()
    return nc


_CACHE = {}


def _get_nc(iters=1):
    if iters not in _CACHE:
        _CACHE[iters] = build_nc(iters)
    return _CACHE[iters]


def _bf16(a):
    import ml_dtypes
    return np.asarray(a, dtype=ml_dtypes.bfloat16)


def _make_rpt(relpos):
    # rpt[p, jc*L + i] = relpos[clip(384 + 128*jc + p - i, 0, 766)]
    j = np.arange(L)[:, None]
    i = np.arange(L)[None, :]
    idx = np.clip(MAXEMBED + j - i, 0, 2 * MAXEMBED - 2)
    rp = relpos[idx]                                  # [j, i]
    return np.ascontiguousarray(
        rp.reshape(JC, 128, L).transpose(1, 0, 2).reshape(128, JC * L))


def _host_prep(inputs):
    f32 = lambda a: np.ascontiguousarray(np.asarray(a), dtype=np.float32)
    x, xorg, abspos = f32(inputs["x"]), f32(inputs["xorg"]), f32(inputs["abspos"])
    mask, norm = f32(inputs["mask"]), f32(inputs["norm"])
    relpos = f32(inputs["relpos"])

    def expand_res(r):
        e = np.repeat(f32(r).reshape(-1), DIMGROUP)          # [512]
        return np.ascontiguousarray(e.reshape(KC, 128).T)    # [128, KC]

    cblob = np.zeros((128, CB_COLS), np.float32)
    cblob[:, CB_I128:CB_I128 + 128] = np.eye(128, dtype=np.float32)
    for h in range(NUMHEAD):
        cblob[:, CB_SEL8 + 8 * h + (h % 2)] = 1.0
    cblob[:, CB_VRES:CB_VRES + KC] = expand_res(inputs["vorg_res"])
    cblob[:, CB_QKO:CB_QKO + KC] = expand_res(inputs["qkorg_res"])
    cblob[:, CB_QKP:CB_QKP + KC] = expand_res(inputs["qkpos_res"])
    cblob[:, CB_DB:CB_DB + KC] = np.ascontiguousarray(
        f32(inputs["dense_b"]).reshape(OC, 128).T)
    cblob[:, CB_ONES:CB_ONES + 64] = 1.0

    blob8 = np.zeros((NUMHEAD, B8_COLS), np.float32)
    for h in range(NUMHEAD):
        blob8[h % 2, B8_BSEL + 64 * h:B8_BSEL + 64 * h + 64] = 1.0
    blob8[:, B8_I8:B8_I8 + NUMHEAD] = np.eye(NUMHEAD, dtype=np.float32)
    blob8[:, B8_GB] = f32(inputs["gate_b"])

    rpt = _make_rpt(relpos)
    iswap = np.zeros((128, 128), np.float32)
    r = np.arange(64)
    iswap[64 + r, r] = 1.0
    iswap[r, 64 + r] = 1.0
    shared = {
        "rptin": _bf16(rpt),
        "rptsw": _bf16(np.concatenate([rpt[64:], rpt[:64]], axis=0)),
        "i128b": _bf16(np.eye(128, dtype=np.float32)),
        "iswapb": _bf16(iswap),
        "gate_wT": _bf16(np.ascontiguousarray(f32(inputs["gate_w"]).T)),
        "q_wT": _bf16(np.ascontiguousarray(f32(inputs["q_w"]).T)),
        "k_wT": _bf16(np.ascontiguousarray(f32(inputs["k_w"]).T)),
        "v_wT": _bf16(np.ascontiguousarray(f32(inputs["v_w"]).T)),
        "dense_wT": _bf16(np.ascontiguousarray(f32(inputs["dense_w"]).T)),
        "cblob": cblob,
        "blob8": blob8,
    }
    # mblob: [N, 128, 4] = [maskT(3) | norm(1)]
    mblob = np.zeros((N, 128, 4), np.float32)
    mblob[:, :, 0:3] = mask.reshape(N, JC, 128).transpose(0, 2, 1)
    mblob[:, :, 3] = norm[:, None]
    xb, xob, abb = _bf16(x), _bf16(xorg), _bf16(abspos)
    in_maps = []
    for c in range(NCORES):
        sl = slice(BPC * c, BPC * (c + 1))
        m = dict(shared)
        m["x"] = xb[sl]
        m["xorg"] = xob[sl]
        m["abspos"] = abb[sl]
        m["mblob"] = mblob[sl]
        in_maps.append(m)
    return in_maps


def run_on_hw(inputs, iters=1):
    nc = _get_nc(iters)
    in_maps = _host_prep(inputs)
    res = run_bass_kernel_spmd(nc, in_maps, list(range(NCORES)))
    return np.concatenate([res.results[c]["out"] for c in range(NCORES)], axis=0)


def kernel(**inputs) -> np.ndarray:
    return run_on_hw(inputs, iters=1)



# revision 25
# speedup vs baseline: 1.2702x; 1.0785x over previous
import sys
sys.path.insert(0, "/opt/trn_rl_repo")
import numpy as np
import concourse.bass as bass
import concourse.bacc as bacc
import concourse.tile as tile
from concourse import mybir
from concourse.bass_utils import run_bass_kernel_spmd
from concourse import bass_isa

F32 = mybir.dt.float32
F32R = mybir.dt.float32r
BF16 = mybir.dt.bfloat16
AF = mybir.ActivationFunctionType
OP = mybir.AluOpType

import os
SPLIT_RELPOS = os.environ.get("SPLIT_RELPOS", "1") == "1"

N, CIO, L = 16, 512, 384
DIMHEAD, NUMHEAD, MAXEMBED, DIMGROUP = 64, 8, 384, 8
CHID = DIMHEAD * NUMHEAD
NCORES = 8
BPC = N // NCORES      # batches per core
KC = CIO // 128        # contraction chunks
OC = CHID // 128       # output-channel chunks
JC = L // 128          # key-position chunks

# constant-blob column layout (128-partition blob, fp32r-typed)
CB_I128 = 0            # [128,128] identity
CB_SEL8 = 128          # [128,64] head-selector columns
CB_VRES = 192          # [128,4]
CB_QKO = 196           # [128,4]
CB_QKP = 200           # [128,4]
CB_DB = 204            # [128,4] dense bias
CB_ONES = 208          # [128,64] all-ones
CB_COLS = 272
# 8-partition blob
B8_BSEL = 0            # [8,512]
B8_I8 = 512            # [8,8]
B8_GB = 520            # [8,1] gate bias
B8_COLS = 521


def build_nc(iters=1):
    nc = bacc.Bacc("TRN2", target_bir_lowering=False, debug=False)

    def inp(name, shape, dt):
        return nc.dram_tensor(name, shape, dt, kind="ExternalInput").ap()

    x = inp("x", [BPC, CIO, L], BF16)
    xorg = inp("xorg", [BPC, CIO, L], BF16)
    abspos = inp("abspos", [BPC, CIO, L], BF16)
    mblob = inp("mblob", [BPC, 128, 4], F32)     # [maskT(3) | norm(1)]
    rptin = inp("rptin", [128, JC * L], BF16)
    rptsw = inp("rptsw", [128, JC * L], BF16)    # partition-halves swapped
    i128b = inp("i128b", [128, 128], BF16)
    iswapb = inp("iswapb", [128, 128], BF16)     # anti-diagonal I64 blocks
    gate_wT = inp("gate_wT", [CIO, NUMHEAD], BF16)
    q_wT = inp("q_wT", [CIO, CHID], BF16)
    k_wT = inp("k_wT", [CIO, CHID], BF16)
    v_wT = inp("v_wT", [CIO, CHID], BF16)
    dense_wT = inp("dense_wT", [CHID, CIO], BF16)
    cblob = inp("cblob", [128, CB_COLS], F32R)
    blob8 = inp("blob8", [NUMHEAD, B8_COLS], F32R)
    out = nc.dram_tensor("out", [BPC, CIO, L], F32, kind="ExternalOutput").ap()

    def chunked_src(t, b):
        # [CIO, L] DRAM slice viewed as [p(128), kc(4), l(384)]
        return bass.AP(tensor=t.tensor, offset=b * CIO * L,
                       ap=[[L, 128], [128 * L, KC], [1, L]])

    def wsrc(t):
        # [CIO, CHID] viewed as [p(128), kc(4), c(CHID)]
        return bass.AP(tensor=t.tensor, offset=0,
                       ap=[[CHID, 128], [128 * CHID, KC], [1, CHID]])

    with tile.TileContext(nc) as tc:
        with tc.tile_pool(name="wts", bufs=1) as wp, \
             tc.tile_pool(name="a2", bufs=2) as a2, \
             tc.tile_pool(name="a1", bufs=1) as a1, \
             tc.tile_pool(name="pp", bufs=12) as ppool, \
             tc.tile_pool(name="ps_big", bufs=2, space="PSUM") as psb, \
             tc.tile_pool(name="ps_s", bufs=2, space="PSUM") as pss, \
             tc.tile_pool(name="ps_o", bufs=2, space="PSUM") as pso:

            first = [True]
            weights = {}

            for it in range(iters):
                binp = []

                def emit_inp(b, ab_engine=None):
                    txa = a1.tile([128, KC * L], BF16, tag=f"xa{b}", name=f"xa{b}")
                    nc.sync.dma_start(out=txa, in_=chunked_src(x, b))
                    txo = a1.tile([128, KC * L], BF16, tag=f"xo{b}", name=f"xo{b}")
                    nc.sync.dma_start(out=txo, in_=chunked_src(xorg, b))
                    tab = a1.tile([128, KC * L], BF16, tag=f"ab{b}", name=f"ab{b}")
                    (ab_engine or nc.gpsimd).dma_start(out=tab,
                                                       in_=chunked_src(abspos, b))
                    binp.append((txa, txo, tab))

                def wdma(nm, shape, dt, srcap):
                    t = wp.tile([128, shape], dt, tag=nm, name=nm)
                    nc.scalar.dma_start(out=t, in_=srcap)
                    weights[nm] = t
                    return t

                if first[0]:
                    w = weights
                    emit_inp(0)
                    t = wp.tile([NUMHEAD, B8_COLS], F32R, tag="b8", name="b8")
                    nc.scalar.dma_start(out=t, in_=blob8)
                    w["b8"] = t
                    wdma("wg", KC * NUMHEAD, BF16,
                         bass.AP(tensor=gate_wT.tensor, offset=0,
                                 ap=[[NUMHEAD, 128], [128 * NUMHEAD, KC],
                                     [1, NUMHEAD]]))
                    wdma("cb", CB_COLS, F32R, cblob)
                    w["mb"] = []
                    for b in range(BPC):
                        t = wp.tile([128, 4], F32, tag=f"mb{b}", name=f"mb{b}")
                        nc.scalar.dma_start(out=t, in_=mblob[b])
                        w["mb"].append(t)
                    wdma("wq", KC * CHID, BF16, wsrc(q_wT))
                    wdma("wk", KC * CHID, BF16, wsrc(k_wT))
                    wdma("i128b", 128, BF16, i128b)
                    wdma("iswapb", 128, BF16, iswapb)
                    rt = wdma("rptw", JC * L, BF16, rptin)
                    w["rpt"] = [rt[:, jc * L:(jc + 1) * L] for jc in range(JC)]
                    rs = wdma("rptsww", JC * L, BF16, rptsw)
                    w["rptsw"] = [rs[:, jc * L:(jc + 1) * L] for jc in range(JC)]
                    wdma("wv", KC * CHID, BF16, wsrc(v_wT))
                    wdma("wd", KC * CHID, BF16, wsrc(dense_wT))
                    emit_inp(1, ab_engine=nc.sync)
                    first[0] = False
                else:
                    emit_inp(0)
                    emit_inp(1)
                w = weights
                cb, b8 = w["cb"], w["b8"]

                for b in range(BPC):
                    txa, txo, tab = binp[b]
                    xs = lambda t, kc: t[:, kc * L:(kc + 1) * L]

                    rn = a2.tile([128, 1], F32, tag="rn", name="rn")
                    nc.vector.reciprocal(rn, w["mb"][b][:, 3:4])

                    # ---- x0 = x + vres*xorg ; x1 = x + qko*xorg + qkp*abspos ----
                    tx0 = a1.tile([128, KC * L], BF16, tag="x0", name="x0")
                    tx1 = a1.tile([128, KC * L], BF16, tag="x1", name="x1")
                    ttm = a1.tile([128, KC * L], F32, tag="tt", name="tt")
                    tmv = a1.tile([128, KC * L], F32, tag="tmv", name="tmv")

                    def bcast1(col):
                        sv = cb[:, col:col + 1].bitcast(F32)
                        return bass.AP(tensor=sv.tensor, offset=sv.offset,
                                       ap=[sv.ap[0], [0, L]])

                    # x1 first (gates q/k projections): DVE fused ops
                    for kc in range(KC):
                        nc.vector.scalar_tensor_tensor(
                            xs(ttm, kc), xs(txo, kc),
                            cb[:, CB_QKO + kc:CB_QKO + kc + 1].bitcast(F32),
                            xs(txa, kc), OP.mult, OP.add)
                        nc.vector.scalar_tensor_tensor(
                            xs(tx1, kc), xs(tab, kc),
                            cb[:, CB_QKP + kc:CB_QKP + kc + 1].bitcast(F32),
                            xs(ttm, kc), OP.mult, OP.add)
                    # x0 (feeds vT later): Pool, off the critical path
                    for kc in range(KC):
                        nc.gpsimd.tensor_tensor(out=xs(tmv, kc), in0=xs(txo, kc),
                                                in1=bcast1(CB_VRES + kc), op=OP.mult)
                        nc.gpsimd.tensor_tensor(out=xs(tx0, kc), in0=xs(tmv, kc),
                                                in1=xs(txa, kc), op=OP.add)

                    # ---- gate projection (PE) ----
                    g_ps = psb.tile([NUMHEAD, L], F32, tag="big", name="g_ps")
                    for kc in range(KC):
                        nc.tensor.matmul(
                            g_ps, w["wg"][:, kc * NUMHEAD:(kc + 1) * NUMHEAD],
                            xs(txa, kc), start=(kc == 0), stop=(kc == KC - 1))
                    tgate = a2.tile([NUMHEAD, L], F32R, tag="gate", name="gate")
                    nc.scalar.activation(tgate, g_ps, AF.Identity,
                                         bias=b8[:, B8_GB:B8_GB + 1].bitcast(F32))

                    # ---- q/k projections (PE busy while gate transposes) ----
                    def proj(wall, rhs_all, nm):
                        dst = a2.tile([128, OC * L], BF16, tag=nm, name=nm)
                        for o in range(OC):
                            ps = psb.tile([128, L], F32, tag="big",
                                          name=f"{nm}p{o}")
                            for kc in range(KC):
                                nc.tensor.matmul(
                                    ps,
                                    wall[:, kc * CHID + o * 128:
                                         kc * CHID + o * 128 + 128],
                                    xs(rhs_all, kc),
                                    start=(kc == 0), stop=(kc == KC - 1))
                            if o == 0:
                                nc.scalar.copy(dst[:, o * L:(o + 1) * L], ps)
                            else:
                                nc.vector.tensor_copy(dst[:, o * L:(o + 1) * L], ps)
                        return dst

                    tgm = []

                    def emit_gt():
                        for jc in range(JC):
                            gt_ps = psb.tile([128, NUMHEAD], F32, tag="big",
                                             name=f"gt{jc}")
                            nc.tensor.matmul(gt_ps,
                                             tgate[:, jc * 128:(jc + 1) * 128],
                                             b8[:, B8_I8:B8_I8 + NUMHEAD],
                                             start=True, stop=True)
                            gm = a2.tile([128, NUMHEAD], F32, tag=f"gm{jc}",
                                         name=f"gm{jc}")
                            nc.vector.tensor_scalar(gm, gt_ps,
                                                    w["mb"][b][:, jc:jc + 1],
                                                    rn[:, 0:1], OP.add, OP.mult)
                            tgm.append(gm)

                    tq = a2.tile([128, OC * L], BF16, tag="q", name="q")
                    for o in range(OC):
                        ps = psb.tile([128, L], F32, tag="big", name=f"qp{o}")
                        for kc in range(KC):
                            nc.tensor.matmul(
                                ps,
                                w["wq"][:, kc * CHID + o * 128:
                                        kc * CHID + o * 128 + 128],
                                xs(tx1, kc),
                                start=(kc == 0), stop=(kc == KC - 1))
                        if o == 0:
                            nc.scalar.copy(tq[:, o * L:(o + 1) * L], ps)
                        else:
                            nc.vector.tensor_copy(tq[:, o * L:(o + 1) * L], ps)
                    emit_gt()

                    tk = proj(w["wk"], tx1, "k")

                    # ---- vT = x0^T @ v_wT ; per-head 65-col blocks, col 64 = ones ----
                    VW = DIMHEAD + 1
                    tvt = a2.tile([128, JC * NUMHEAD * VW], F32R, tag="vt", name="vt")

                    for lc in range(JC):
                        ps = psb.tile([128, CHID], F32, tag="big", name=f"vtp{lc}")
                        for kc in range(KC):
                            nc.tensor.matmul(
                                ps, tx0[:, kc * L + lc * 128:kc * L + lc * 128 + 128],
                                w["wv"][:, kc * CHID:(kc + 1) * CHID],
                                start=(kc == 0), stop=(kc == KC - 1))
                        blk = tvt[:, lc * NUMHEAD * VW:(lc + 1) * NUMHEAD * VW]
                        blk = blk.rearrange("p (h c) -> p h c", c=VW)
                        nc.vector.tensor_copy(blk[:, :, 0:DIMHEAD], ps)
                        nc.vector.tensor_copy(blk[:, :, DIMHEAD:VW],
                                              cb[:, CB_ONES:CB_ONES + NUMHEAD])

                    # ---- attention (emission software-pipelined over head pairs) ----
                    # QK pairs run concurrently on disjoint PE row groups;
                    # relpos pre-fill uses 4 quadrant-disjoint 64-row matmuls
                    # (rpt + partition-swapped rptsw) so all 4 overlap on HW.
                    tP_heads = {}
                    tatt = a1.tile([128, OC * L], BF16, tag="att", name="att")

                    def emit_qk_pair(hp):
                        he, ho = 2 * hp, 2 * hp + 1
                        tPe, tPo = [], []
                        for jc in range(JC):
                            # one 2-bank psum tile per pair: even head in bank
                            # 0, odd head in bank 1
                            s2 = pss.tile([128, 1024], F32, tag="s",
                                          name=f"s{hp}j{jc}")
                            s_e, s_o = s2[:, 0:L], s2[:, 512:512 + L]
                            rpt, rsw = w["rpt"][jc], w["rptsw"][jc]
                            ib, isw = w["i128b"], w["iswapb"]
                            if SPLIT_RELPOS:
                                nc.tensor.matmul(s_e[0:64, :], ib[0:64, 0:64],
                                                 rpt[0:64, :], start=True,
                                                 stop=False, skip_group_check=True)
                                nc.tensor.matmul(s_e[64:128, :],
                                                 ib[64:128, 64:128],
                                                 rpt[64:128, :], start=True,
                                                 stop=False, skip_group_check=True)
                                nc.tensor.matmul(s_o[0:64, :], isw[64:128, 0:64],
                                                 rsw[64:128, :], start=True,
                                                 stop=False, skip_group_check=True)
                                nc.tensor.matmul(s_o[64:128, :], isw[0:64, 64:128],
                                                 rsw[0:64, :], start=True,
                                                 stop=False, skip_group_check=True)
                            else:
                                nc.tensor.matmul(s_e, ib, rpt, start=True,
                                                 stop=False, skip_group_check=True)
                                nc.tensor.matmul(s_o, ib, rpt, start=True,
                                                 stop=False, skip_group_check=True)
                            co = hp * L + jc * 128
                            nc.tensor.matmul(s_e, tk[0:64, co:co + 128],
                                             tq[0:64, hp * L:(hp + 1) * L],
                                             start=False, stop=True,
                                             skip_group_check=True)
                            nc.tensor.matmul(s_o, tk[64:128, co:co + 128],
                                             tq[64:128, hp * L:(hp + 1) * L],
                                             start=False, stop=True,
                                             skip_group_check=True)
                            Pe = ppool.tile([128, L], F32R, tag="P",
                                            name=f"P{he}j{jc}")
                            nc.scalar.activation(Pe, s_e, AF.Exp,
                                                 bias=tgm[jc][:, he:he + 1],
                                                 scale=rn[:, 0:1])
                            Po = ppool.tile([128, L], F32R, tag="P",
                                            name=f"P{ho}j{jc}")
                            nc.scalar.activation(Po, s_o, AF.Exp,
                                                 bias=tgm[jc][:, ho:ho + 1],
                                                 scale=rn[:, 0:1])
                            tPe.append(Pe)
                            tPo.append(Po)
                        tP_heads[he] = tPe
                        tP_heads[ho] = tPo

                    def emit_av_pair(hp):
                        he, ho = 2 * hp, 2 * hp + 1
                        oUs = {}
                        for h in (he, ho):
                            tP = tP_heads.pop(h)
                            o_ps = pso.tile([VW, L], F32, tag="o", name=f"o{h}")
                            for jc in range(JC):
                                nc.tensor.matmul(
                                    o_ps,
                                    tvt[:, jc * NUMHEAD * VW + VW * h:
                                        jc * NUMHEAD * VW + VW * h + VW],
                                    tP[jc], start=(jc == 0), stop=(jc == JC - 1),
                                    skip_group_check=True)
                            oU = a1.tile([VW, L], F32, tag=f"oU{h}", name=f"oU{h}")
                            if h >= NUMHEAD - 2:
                                nc.scalar.copy(oU, o_ps)
                            else:
                                nc.vector.tensor_copy(oU, o_ps)
                            rcpr65 = a2.tile([VW, L], F32R, tag="rcpr65",
                                             name=f"rr{h}")
                            with nc.allow_low_precision(reason="fp32r recip"):
                                nc.vector.reciprocal(rcpr65[64:65, :], oU[64:65, :])
                            oUs[h] = (oU, rcpr65)
                        for h, off in ((he, 0), (ho, 64)):
                            rb_ps = psb.tile([64, L], F32, tag="big",
                                             name=f"rb{h}")
                            nc.tensor.matmul(rb_ps,
                                             cb[64:65, CB_ONES:CB_ONES + 64],
                                             oUs[h][1][64:65, :], start=True,
                                             stop=True, skip_group_check=True)
                            nc.vector.tensor_tensor(
                                out=tatt[off:off + 64, hp * L:(hp + 1) * L],
                                in0=oUs[h][0][0:64, :], in1=rb_ps,
                                op=OP.mult)

                    emit_qk_pair(0)
                    tout = a2.tile([128, OC * L], F32, tag="outt", name="outt")
                    d_ps = {}

                    def emit_dense_partial(o, kcs, stop):
                        if o not in d_ps:
                            d_ps[o] = psb.tile([128, L], F32, tag="big",
                                               name=f"dp{o}")
                        for kc in kcs:
                            nc.tensor.matmul(
                                d_ps[o],
                                w["wd"][:, kc * CIO + o * 128:kc * CIO + o * 128 + 128],
                                tatt[:, kc * L:(kc + 1) * L],
                                start=(kc == 0), stop=(stop and kc == kcs[-1]),
                                skip_group_check=True)
                        if stop:
                            nc.scalar.activation(
                                tout[:, o * L:(o + 1) * L], d_ps.pop(o), AF.Identity,
                                bias=cb[:, CB_DB + o:CB_DB + o + 1].bitcast(F32))

                    for hp in range(NUMHEAD // 2):
                        if hp + 1 < NUMHEAD // 2:
                            emit_qk_pair(hp + 1)
                        emit_av_pair(hp)
                        if hp == 2:
                            emit_dense_partial(0, [0, 1, 2], stop=False)
                    emit_dense_partial(0, [3], stop=True)
                    for o in (1, 2, 3):
                        emit_dense_partial(o, [0, 1, 2, 3], stop=True)
                    for half in range(2):
                        dst = bass.AP(tensor=out.tensor,
                                      offset=b * CIO * L + half * 2 * 128 * L,
                                      ap=[[L, 128], [128 * L, 2], [1, L]])
                        nc.sync.dma_start(
                            out=dst, in_=tout[:, half * 2 * L:(half + 1) * 2 * L])

    nc.compile()
    return nc


_CACHE = {}


def _get_nc(iters=1):
    if iters not in _CACHE:
        _CACHE[iters] = build_nc(iters)
    return _CACHE[iters]


def _bf16(a):
    import ml_dtypes
    return np.asarray(a, dtype=ml_dtypes.bfloat16)


def _make_rpt(relpos):
    # rpt[p, jc*L + i] = relpos[clip(384 + 128*jc + p - i, 0, 766)]
    j = np.arange(L)[:, None]
    i = np.arange(L)[None, :]
    idx = np.clip(MAXEMBED + j - i, 0, 2 * MAXEMBED - 2)
    rp = relpos[idx]                                  # [j, i]
    return np.ascontiguousarray(
        rp.reshape(JC, 128, L).transpose(1, 0, 2).reshape(128, JC * L))


def _host_prep(inputs):
    f32 = lambda a: np.ascontiguousarray(np.asarray(a), dtype=np.float32)
    x, xorg, abspos = f32(inputs["x"]), f32(inputs["xorg"]), f32(inputs["abspos"])
    mask, norm = f32(inputs["mask"]), f32(inputs["norm"])
    relpos = f32(inputs["relpos"])

    def expand_res(r):
        e = np.repeat(f32(r).reshape(-1), DIMGROUP)          # [512]
        return np.ascontiguousarray(e.reshape(KC, 128).T)    # [128, KC]

    cblob = np.zeros((128, CB_COLS), np.float32)
    cblob[:, CB_I128:CB_I128 + 128] = np.eye(128, dtype=np.float32)
    for h in range(NUMHEAD):
        cblob[:, CB_SEL8 + 8 * h + (h % 2)] = 1.0
    cblob[:, CB_VRES:CB_VRES + KC] = expand_res(inputs["vorg_res"])
    cblob[:, CB_QKO:CB_QKO + KC] = expand_res(inputs["qkorg_res"])
    cblob[:, CB_QKP:CB_QKP + KC] = expand_res(inputs["qkpos_res"])
    cblob[:, CB_DB:CB_DB + KC] = np.ascontiguousarray(
        f32(inputs["dense_b"]).reshape(OC, 128).T)
    cblob[:, CB_ONES:CB_ONES + 64] = 1.0

    blob8 = np.zeros((NUMHEAD, B8_COLS), np.float32)
    for h in range(NUMHEAD):
        blob8[h % 2, B8_BSEL + 64 * h:B8_BSEL + 64 * h + 64] = 1.0
    blob8[:, B8_I8:B8_I8 + NUMHEAD] = np.eye(NUMHEAD, dtype=np.float32)
    blob8[:, B8_GB] = f32(inputs["gate_b"])

    rpt = _make_rpt(relpos)
    iswap = np.zeros((128, 128), np.float32)
    r = np.arange(64)
    iswap[64 + r, r] = 1.0
    iswap[r, 64 + r] = 1.0
    shared = {
        "rptin": _bf16(rpt),
        "rptsw": _bf16(np.concatenate([rpt[64:], rpt[:64]], axis=0)),
        "i128b": _bf16(np.eye(128, dtype=np.float32)),
        "iswapb": _bf16(iswap),
        "gate_wT": _bf16(np.ascontiguousarray(f32(inputs["gate_w"]).T)),
        "q_wT": _bf16(np.ascontiguousarray(f32(inputs["q_w"]).T)),
        "k_wT": _bf16(np.ascontiguousarray(f32(inputs["k_w"]).T)),
        "v_wT": _bf16(np.ascontiguousarray(f32(inputs["v_w"]).T)),
        "dense_wT": _bf16(np.ascontiguousarray(f32(inputs["dense_w"]).T)),
        "cblob": cblob,
        "blob8": blob8,
    }
    # mblob: [N, 128, 4] = [maskT(3) | norm(1)]
    mblob = np.zeros((N, 128, 4), np.float32)
    mblob[:, :, 0:3] = mask.reshape(N, JC, 128).transpose(0, 2, 1)
    mblob[:, :, 3] = norm[:, None]
    xb, xob, abb = _bf16(x), _bf16(xorg), _bf16(abspos)
    in_maps = []
    for c in range(NCORES):
        sl = slice(BPC * c, BPC * (c + 1))
        m = dict(shared)
        m["x"] = xb[sl]
        m["xorg"] = xob[sl]
        m["abspos"] = abb[sl]
        m["mblob"] = mblob[sl]
        in_maps.append(m)
    return in_maps


def run_on_hw(inputs, iters=1):
    nc = _get_nc(iters)
    in_maps = _host_prep(inputs)
    res = run_bass_kernel_spmd(nc, in_maps, list(range(NCORES)))
    return np.concatenate([res.results[c]["out"] for c in range(NCORES)], axis=0)


def kernel(**inputs) -> np.ndarray:
    return run_on_hw(inputs, iters=1)

